# revision 2
# baseline (speedup 1.0000x reference)
"""FP8 block-wise dequant linear: out[b,s,o] = sum_i x[b,s,i] * (w[o,i]*scale[o//128,i//128]).

Sharding: 4-way over seq x 2-way over out_features across 8 NeuronCores.
Per core: x shard [512 seq, 4096 in] (bf16, host-precast), w shard
[2048 out, 4096 in] fp8, out [512, 2048] (bf16 on device, f32 on host).

Device pipeline per core (stationary-w orientation, out[o, s]):
  - x [k-on-partitions, kb*512+s] DMAs into a double-buffered resident
    SBUF tile (4 x 1 MiB pieces so the first matmul after a loop
    back-edge only waits ~3 us).
  - w fp8 is streamed per ob-block group; DVE dequantizes each ob piece
    [p, kb*128+o] to bf16 in ONE tensor_tensor with a per-kb broadcast
    scale (3D AP). Each ob piece is read only during its own 1/16 of the
    matmul phase, so next iteration's w stream overlaps this iteration's
    compute at Tile's subregion granularity.
  - TensorE: for each ob: 32 accumulating matmuls, stationary wq block
    [k128, o128], moving x slab [k128, s512], PSUM bank rotation (4),
    ScalarE evacuates psum -> bf16, DMA out.
  - Timing loop: For_i over UNROLL unrolled bodies + hint_engines=(PE,)
    so the ~2us back-edge barrier + IRAM refetch amortize.

Roofline: 512 N=512 bf16 matmuls/core = 109.2 us PE floor; total DMA
14.25 MiB/body = ~42 us at 358 GB/s (fully hidden).
"""

import numpy as np
import ml_dtypes

import concourse.bacc as bacc
import concourse.mybir as mybir
from concourse.tile import TileContext
from concourse.bass_utils import run_bass_kernel_spmd

SEQ, DIN, DOUT = 2048, 4096, 4096
N_CORES = 8
SEQ_SHARDS, OUT_SHARDS = 4, 2
SEQ_SH, OUT_SH = SEQ // SEQ_SHARDS, DOUT // OUT_SHARDS  # 512, 2048
P = 128
NKB = DIN // P            # 32 contraction blocks
NOB = OUT_SH // P         # 16 out blocks per core
NMM = SEQ_SH              # 512 moving free dim (= one PSUM bank)
OBW = NKB * P             # 4096 w cols per ob piece

XPIECES = 4               # x DMA pieces (1 MiB each)
WGROUPS = [[0], [1, 2, 3], [4, 5, 6, 7], [8, 9, 10, 11], [12, 13, 14, 15]]
UNROLL = 4                # bodies per For_i iteration


def emit_body(nc, pools, io):
    dt = mybir.dt
    x_pool, wf_pool, wq_pool, sc_pool, ps_pool, ob_pool = pools
    xt, wt, sc, out = io

    sc_sb = sc_pool.tile([P, NOB * NKB], dt.float32, tag="sc")
    nc.sync.dma_start(sc_sb[:], sc[:])

    xb = x_pool.tile([P, NKB * SEQ_SH], dt.bfloat16, tag="xb")
    xstep = (NKB // XPIECES) * SEQ_SH
    for i in range(XPIECES):
        nc.gpsimd.dma_start(xb[:, i * xstep:(i + 1) * xstep],
                            xt[:, i * xstep:(i + 1) * xstep])

    wf_tiles = {}
    for g in WGROUPS:
        wf = wf_pool.tile([P, 4 * OBW], dt.float8e4, tag="wf")
        nc.gpsimd.dma_start(wf[:, :len(g) * OBW],
                            wt[:, g[0] * OBW:(g[-1] + 1) * OBW])
        for j, ob in enumerate(g):
            wf_tiles[ob] = wf[:, j * OBW:(j + 1) * OBW]

    for ob in range(NOB):
        wq = wq_pool.tile([P, OBW], dt.bfloat16, tag="wq")
        s_b = (sc_sb[:, ob * NKB:(ob + 1) * NKB]
               .unsqueeze(2).broadcast_to([P, NKB, P]))
        nc.vector.tensor_mul(
            wq.rearrange("p (k o) -> p k o", k=NKB),
            wf_tiles[ob].rearrange("p (k o) -> p k o", k=NKB),
            s_b,
        )
        ps = ps_pool.tile([P, NMM], dt.float32, tag="ps")
        for kb in range(NKB):
            nc.tensor.matmul(
                ps[:],
                wq[:, kb * P:(kb + 1) * P],
                xb[:, kb * SEQ_SH:(kb + 1) * SEQ_SH],
                start=(kb == 0),
                stop=(kb == NKB - 1),
            )
        obt = ob_pool.tile([P, NMM], dt.bfloat16, tag="ob")
        nc.scalar.copy(obt[:], ps[:])
        nc.gpsimd.dma_start(out[:, ob * NMM:(ob + 1) * NMM], obt[:])


def build_nc(iters=1, loop=None, **kw):
    nc = bacc.Bacc(None, target_bir_lowering=False)
    xt = nc.dram_tensor("xt", [P, NKB * SEQ_SH], mybir.dt.bfloat16, kind="ExternalInput")
    wt = nc.dram_tensor("wt", [P, NOB * OBW], mybir.dt.float8e4, kind="ExternalInput")
    sc = nc.dram_tensor("sc", [P, NOB * NKB], mybir.dt.float32, kind="ExternalInput")
    out = nc.dram_tensor("out", [P, NOB * NMM], mybir.dt.bfloat16,
                         kind="ExternalOutput")
    io = (xt, wt, sc, out)

    with TileContext(nc) as tc:
        with (
            tc.tile_pool(name="x", bufs=2) as x_pool,
            tc.tile_pool(name="wf", bufs=2) as wf_pool,
            tc.tile_pool(name="wq", bufs=3) as wq_pool,
            tc.tile_pool(name="scp", bufs=2) as sc_pool,
            tc.tile_pool(name="ps", bufs=4, space="PSUM") as ps_pool,
            tc.tile_pool(name="ob", bufs=3) as ob_pool,
        ):
            pools = (x_pool, wf_pool, wq_pool, sc_pool, ps_pool, ob_pool)
            if loop is not None:
                assert loop % UNROLL == 0, (loop, UNROLL)
                with tc.For_i(0, loop // UNROLL, 1,
                              hint_engines=(mybir.EngineType.PE,)):
                    for _ in range(UNROLL):
                        emit_body(nc, pools, io)
            else:
                for _ in range(iters):
                    emit_body(nc, pools, io)
    nc.compile()
    return nc


def shard_inputs(x, weight, weight_scale_inv):
    """Host staging, partition-major per core:
       xt[p, kb*SEQ_SH+s] = x[0][si*SEQ_SH+s, kb*128+p]           (bf16)
       wt[p, ob*OBW+kb*P+o] = w8[oi*OUT_SH+ob*128+o, kb*128+p]    (fp8)
       sc[p, ob*NKB+kb]   = scale[oi*NOB+ob, kb]                  (f32)"""
    x = np.asarray(x)
    weight = np.asarray(weight)
    scale = np.asarray(weight_scale_inv, dtype=np.float32)
    w8 = weight.view(np.uint8)

    in_maps = []
    x_dev = {}
    w_dev = {}
    for c in range(N_CORES):
        si, oi = c % SEQ_SHARDS, c // SEQ_SHARDS
        if si not in x_dev:
            xs = np.asarray(x[0][si * SEQ_SH:(si + 1) * SEQ_SH, :],
                            dtype=np.float32).astype(ml_dtypes.bfloat16)
            x_dev[si] = np.ascontiguousarray(
                xs.T.reshape(NKB, P, SEQ_SH).transpose(1, 0, 2)
            ).reshape(P, NKB * SEQ_SH)
        if oi not in w_dev:
            ws = w8[oi * OUT_SH:(oi + 1) * OUT_SH, :]   # [(ob,o), (kb,p)]
            w_dev[oi] = np.ascontiguousarray(
                ws.reshape(NOB, P, NKB, P).transpose(3, 0, 2, 1)
            ).reshape(P, NOB * OBW).view(ml_dtypes.float8_e4m3)
        sc_core = scale[oi * NOB:(oi + 1) * NOB, :]     # [NOB, NKB]
        sc_st = np.ascontiguousarray(
            np.broadcast_to(sc_core.reshape(1, NOB * NKB), (P, NOB * NKB)))
        in_maps.append({"xt": x_dev[si], "wt": w_dev[oi], "sc": sc_st})
    return in_maps


def unshard_output(results):
    out = np.empty((1, SEQ, DOUT), dtype=np.float32)
    for c in range(N_CORES):
        si, oi = c % SEQ_SHARDS, c // SEQ_SHARDS
        o = np.asarray(results[c]["out"], dtype=np.float32)
        # out_dram[p, ob*NMM+s] = y[s, ob*128+p] -> y[s, (ob,p)]
        y = o.reshape(P, NOB, NMM).transpose(2, 1, 0).reshape(SEQ_SH, OUT_SH)
        out[0, si * SEQ_SH:(si + 1) * SEQ_SH,
            oi * OUT_SH:(oi + 1) * OUT_SH] = y
    return out


_NC_CACHE = {}


def _run_spmd(nc, in_maps, tries=3):
    """The axon-tunneled device occasionally faults with
    NRT_EXEC_UNIT_UNRECOVERABLE, which poisons the whole PJRT client —
    reset jax backends before retrying."""
    import time as _time
    last = None
    for t in range(tries):
        try:
            return run_bass_kernel_spmd(nc, in_maps, core_ids=list(range(N_CORES)))
        except Exception as e:  # noqa: BLE001
            last = e
            _time.sleep(2.0)
            try:
                import jax as _jax
                _jax.clear_backends()
            except Exception:  # noqa: BLE001
                pass
    raise last


def kernel(x, weight, weight_scale_inv):
    if "nc" not in _NC_CACHE:
        _NC_CACHE["nc"] = build_nc()
    nc = _NC_CACHE["nc"]
    in_maps = shard_inputs(x, weight, weight_scale_inv)
    res = _run_spmd(nc, in_maps)
    return unshard_output(res.results)
